# revision 35
# baseline (speedup 1.0000x reference)
"""Multi-head attention forward on 8 TRN2 NeuronCores — v2.

Problem: x[2,2048,1024] @ {Wq,Wk,Wv}[1024,1024] (+bias) -> 16 heads of 64,
softmax(QK^T/8)V per head, concat -> @Wo[1024,1024] + bo.

Sharding: tensor-parallel over d_hid. Core c owns 2 heads (128 dims).
Host sums the 8 partial out projections and adds bo.

v2 design vs v1 (296788ns baseline):
  - Act engine is the wall (128 exps of [128,1024] ~ 165us): everything
    else is scheduled to hide under it via an explicit global slot
    schedule (one slot per (batch, qchunk, ktile) score+exp step).
  - scores: two K=64 matmuls per (qc, ki) (no zero-padded Q tiles).
  - ctx: fp16 [65, 512] matmuls per (ki, head); the 65th weight column
    is ones so psum row 64 accumulates the softmax denominator.
    (fp8 DoubleRow was 2x faster on paper but walrus only accepts DR
    weights that are contiguous [K, 2, M] with M in {32,64,128} at
    column position 0 — no room for the denominator row, and a
    separate den matmul needs 2 PSUM banks we don't have.)
  - out projection reads on-device-normalized ctxT (f16), partials out
    in f16; the 8-way partial sum + bo stays on host.
  - all DMA issuance on sync/gpsimd queues (Act queue = exps only).
  - x^T loaded once for both batches (8MB SBUF resident).
  - PSUM: sc[128,1024]x2 (4 banks) + ctx [65,512]x2 (2) + pp ring (2).
"""

import os
import numpy as np

B, S, D = 2, 2048, 1024
NCORES = 8
HSLICE = D // NCORES          # 128 = 2 heads x 64
KT_PROJ = 8                   # d_in contraction tiles for projections
QH = 512                      # q chunk
NQC = S // QH                 # 4 q chunks per batch
NKT = S // 128                # 16 k tiles per batch
NPAIR = NKT // 2              # 8 ki pairs (fp8 DoubleRow)

_cache = {}


def _build():
    import concourse.bacc as bacc
    import concourse.tile as tile
    from concourse import mybir

    f32 = mybir.dt.float32
    f16 = mybir.dt.float16
    AF = mybir.ActivationFunctionType

    nc = bacc.Bacc("TRN2", target_bir_lowering=False, debug=False,
                   num_devices=NCORES)

    xt_d = nc.dram_tensor("xt", [D, B * S], f16, kind="ExternalInput").ap()
    # wq/wk/wv pre-rearranged on host to [128, 8*128] (k-tiles side by
    # side) so each loads with ONE fast contiguous descriptor.
    wq_d = nc.dram_tensor("wq", [128, D], f16, kind="ExternalInput").ap()
    wk_d = nc.dram_tensor("wk", [128, D], f16, kind="ExternalInput").ap()
    wv_d = nc.dram_tensor("wv", [128, D], f16, kind="ExternalInput").ap()
    bq_d = nc.dram_tensor("bq", [HSLICE, 1], f32, kind="ExternalInput").ap()
    bk_d = nc.dram_tensor("bk", [HSLICE, 1], f32, kind="ExternalInput").ap()
    bv_d = nc.dram_tensor("bv", [HSLICE, 1], f32, kind="ExternalInput").ap()
    wo_d = nc.dram_tensor("wo", [HSLICE, D], f16, kind="ExternalInput").ap()
    idt_d = nc.dram_tensor("idt", [128, 128], f16, kind="ExternalInput").ap()
    out_d = nc.dram_tensor("out", [B * S, D], f16, kind="ExternalOutput").ap()

    with tile.TileContext(nc) as tc:
        with (
            tc.tile_pool(name="wpool", bufs=1) as wpool,
            tc.tile_pool(name="xtp", bufs=1) as xtp,
            tc.tile_pool(name="qk", bufs=2) as qkp,
            tc.tile_pool(name="vap", bufs=2) as vap,
            tc.tile_pool(name="etp", bufs=6) as etp,
            tc.tile_pool(name="ctxp", bufs=2) as ctxp,
            tc.tile_pool(name="stp", bufs=2) as stp,
            tc.tile_pool(name="normp", bufs=2) as normp,
            tc.tile_pool(name="ostp", bufs=4) as ostp,
            tc.tile_pool(name="psS", bufs=2, space="PSUM") as psS,
            tc.tile_pool(name="psC", bufs=1, space="PSUM") as psC,
            tc.tile_pool(name="psP", bufs=2, space="PSUM") as psP,
        ):
            # ---- weights / constants: ONE descriptor per matrix (3D AP
            # gathers the 8 k-tiles side by side) so startup isn't gated
            # on ~30 serial descriptor issues. gpsimd: wq, wv, idt, wo;
            # scalar: wk (2 descriptors, done long before the first
            # ACTIVATE is reached). ----
            def wtiles(tag, src, eng, bias_d, bias_tag):
                t = wpool.tile([128, KT_PROJ * HSLICE], f16, tag=tag,
                               name=tag)
                eng.dma_start(t[:], src[:])
                b_t = wpool.tile([128, 1], f32, tag=bias_tag, name=bias_tag)
                eng.dma_start(b_t[:], bias_d[:])
                lst = [t[:, ki * HSLICE:(ki + 1) * HSLICE]
                       for ki in range(KT_PROJ)]
                return lst, b_t

            wq_t, bq_t = wtiles("wq", wq_d, nc.gpsimd, bq_d, "bq")
            wk_t, bk_t = wtiles("wk", wk_d, nc.scalar, bk_d, "bk")
            idt = wpool.tile([128, 128], f16, tag="idt")
            nc.gpsimd.dma_start(idt[:], idt_d[:])
            wo_t = wpool.tile([128, D], f16, tag="wo")
            nc.gpsimd.dma_start(wo_t[:], wo_d[:])
            wv_t, bv_t = wtiles("wv", wv_d, nc.gpsimd, bv_d, "bv")

            # ---- x^T, both batches, loaded once. Batch 0 as one 512KB
            # descriptor per k-tile, alternating sync/scalar queues so two
            # DMA rings run in parallel (~2us effective per chunk); batch 1
            # on gpsimd right after the weights (needed only at ~70us). ----
            xts = []
            for ki in range(KT_PROJ):
                t = xtp.tile([128, B * S], f16, tag=f"xt{ki}", name=f"xt{ki}")
                xts.append(t)
            for ki in range(KT_PROJ):
                eng = nc.sync if ki % 2 == 0 else nc.scalar
                eng.dma_start(xts[ki][:, 0:S],
                              xt_d[ki * 128:(ki + 1) * 128, 0:S])
            for ki in range(KT_PROJ):      # batch 1 in one go
                nc.gpsimd.dma_start(
                    xts[ki][:, S:2 * S],
                    xt_d[ki * 128:(ki + 1) * 128, S:2 * S])

            # ---- per-batch tile state ----
            qt = [{} for _ in range(B)]     # qc -> [128, 512] f16
            kt = [{} for _ in range(B)]     # c  -> [128, 512] f16
            vt = [{} for _ in range(B)]     # c  -> [128, 512] f16
            va = [{} for _ in range(B)]     # ki -> [128, 130] f16 (V^T + ones)
            et = [{} for _ in range(B)]     # (qc, p) -> [128, 2048] f16
            ctx_ps = [{} for _ in range(B)]  # (qc, h) -> [65, 512] f32 psum
            stg = [{} for _ in range(B)]    # (qc, h) -> [65, 512] f32
            ctxT = [{} for _ in range(B)]   # qc -> [128, 512] f16

            def proj_step(b, which, c):
                """One projection chunk: 8 matmuls + DVE drain w/ bias."""
                ps = psP.tile([128, 512], f32, tag="pp", name="pp")
                w_t = {"q": wq_t, "k": wk_t, "v": wv_t}[which]
                col0 = b * S + c * 512
                for ki in range(KT_PROJ):
                    nc.tensor.matmul(ps[:], w_t[ki][:],
                                     xts[ki][:, col0:col0 + 512],
                                     start=(ki == 0), stop=(ki == KT_PROJ - 1))
                if which == "q":
                    dst = qkp.tile([128, 512], f16, tag=f"qt{c}", name=f"qt{c}")
                    qt[b][c] = dst
                    b_t = bq_t
                elif which == "k":
                    dst = qkp.tile([128, 512], f16, tag=f"kt{c}", name=f"kt{c}")
                    kt[b][c] = dst
                    b_t = bk_t
                else:
                    dst = qkp.tile([128, 512], f16, tag=f"vt{c}", name=f"vt{c}")
                    vt[b][c] = dst
                    b_t = bv_t
                nc.vector.tensor_scalar_add(dst[:], ps[:], b_t[:, 0:1])

            def vaug_step(b, p):
                """Transpose V tiles ki=2p,2p+1 into f16 [128, 130] va tiles:
                per head h a [65]-col block = 64 V^T dims + a ones column
                (psum row 64 of the ctx matmul = softmax denominator)."""
                for j in range(2):
                    ki = 2 * p + j
                    c = ki // 4
                    vat = vap.tile([128, 130], f16, tag=f"va{ki}",
                                   name=f"va{ki}")
                    va[b][ki] = vat
                    ones_v = vat[:].rearrange("p (h m) -> p h m", h=2)
                    nc.gpsimd.memset(ones_v[:, :, 64:65], 1.0)
                    tp = psP.tile([128, 128], f16, tag="pp", name="tp")
                    nc.tensor.transpose(
                        tp[:], vt[b][c][:, (ki % 4) * 128:(ki % 4 + 1) * 128],
                        idt[:])
                    src = tp[:].rearrange("p (h m) -> p h m", h=2)
                    dstv = vat[:].rearrange(
                        "p (h m) -> p h m", h=2)[:, :, 0:64]
                    nc.vector.tensor_copy(dstv, src)

            def score_step(b, qc, ki):
                sc = psS.tile([128, 1024], f32, tag="sc", name="sc")
                c, kk = ki // 4, (ki % 4) * 128
                for h in range(2):
                    nc.tensor.matmul(
                        sc[:, h * 512:(h + 1) * 512],
                        kt[b][c][h * 64:(h + 1) * 64, kk:kk + 128],
                        qt[b][qc][h * 64:(h + 1) * 64, :],
                        start=True, stop=True)
                p, j = ki // 2, ki % 2
                if j == 0:
                    et[b][(qc, p)] = etp.tile([128, 2048], f16, tag="et",
                                              name="et")
                nc.scalar.activation(
                    et[b][(qc, p)][:, j * 1024:(j + 1) * 1024], sc[:], AF.Exp)

            def ctx_step(b, qc, p):
                # et tile [128, 2048] = [j0: h0|h1, j1: h0|h1] f16.
                ett = et[b][(qc, p)]
                if p == 0:
                    for h in range(2):
                        ctx_ps[b][(qc, h)] = psC.tile([65, 512], f32,
                                                      tag=f"c{h}", name=f"c{h}")
                for j in range(2):
                    ki = 2 * p + j
                    for h in range(2):
                        nc.tensor.matmul(
                            ctx_ps[b][(qc, h)][:],
                            va[b][ki][:, h * 65:(h + 1) * 65],
                            ett[:, j * 1024 + h * 512:j * 1024 + (h + 1) * 512],
                            start=(ki == 0), stop=(ki == NKT - 1))

            def stage_step(b, qc):
                """Drain ctx psum (frees psC fast) + kick off denom path."""
                for h in range(2):
                    st = stp.tile([65, 512], f32, tag=f"st{h}", name=f"st{h}")
                    stg[b][(qc, h)] = st
                    nc.vector.tensor_copy(st[:], ctx_ps[b][(qc, h)][0:65, :])

            bc_t = [{} for _ in range(B)]   # (qc, h) -> [64, 512] f32

            def normA_step(b, qc):
                """Reciprocal + broadcast of the denominators. Split from
                the muls so the DVE's in-order queue never sits waiting on
                the gpsimd broadcast round-trip (that stall delayed outp
                drains and showed up as 7us PE psum-ring waits)."""
                for h in range(2):
                    st = stg[b][(qc, h)]
                    r0 = normp.tile([1, 512], f32, tag=f"r0{h}", name=f"r0{h}")
                    nc.gpsimd.dma_start(r0[:], st[64:65, :])
                    rc = normp.tile([1, 512], f32, tag=f"rc{h}", name=f"rc{h}")
                    nc.vector.reciprocal_approx_fast(rc[:], r0[:])
                    bc = normp.tile([64, 512], f32, tag=f"bc{h}", name=f"bc{h}")
                    nc.gpsimd.partition_broadcast(bc[:], rc[:])
                    bc_t[b][(qc, h)] = bc

            def normB_step(b, qc):
                t = ctxp.tile([128, 512], f16, tag=f"ctxT{qc}",
                              name=f"ctxT{qc}")
                ctxT[b][qc] = t
                for h in range(2):
                    nc.vector.tensor_mul(
                        out=t[h * 64:(h + 1) * 64, :],
                        in0=stg[b][(qc, h)][0:64, :], in1=bc_t[b][(qc, h)][:])

            def outp_step(b, qc, st_i, tail=False):
                row0 = b * S + qc * 512 + st_i * 128
                for half in range(2):
                    # in the tail the score psum banks are free: alternate
                    # pools to deepen the mm->cast round-trip pipeline.
                    if tail and half == 1:
                        po = psS.tile([128, 512], f32, tag="sc", name="po")
                    else:
                        po = psP.tile([128, 512], f32, tag="pp", name="po")
                    nc.tensor.matmul(
                        po[:],
                        ctxT[b][qc][:, st_i * 128:(st_i + 1) * 128],
                        wo_t[:, half * 512:(half + 1) * 512],
                        start=True, stop=True)
                    ot = ostp.tile([128, 512], f16, tag="ost", name="ost")
                    nc.vector.tensor_copy(ot[:], po[:])
                    eng = nc.sync if half == 0 else nc.gpsimd
                    eng.dma_start(
                        out_d[row0:row0 + 128, half * 512:(half + 1) * 512],
                        ot[:])

            # ---- global slot schedule ----
            from collections import defaultdict
            actions = defaultdict(list)   # g -> [(prio, fn)]

            # slot priorities: scores(0) feed the Act engine (the wall);
            # stage(1) frees ctx psum BEFORE the next qchunk's first ctx
            # allocates it (prio 3 > 1 at the shared slot); then outp(4)
            # and proj/vaug fillers(5).
            for b in range(B):
                base = b * 64
                for qc in range(NQC):
                    tail = (b == B - 1 and qc == NQC - 1)
                    for ki in range(NKT):
                        g = base + qc * 16 + ki
                        actions[g].append(
                            (0, (lambda b=b, qc=qc, ki=ki:
                                 score_step(b, qc, ki))))
                    for p in range(NPAIR):
                        # compress the very last qchunk: no exps pace the
                        # tail, and HAM tends to run it at half speed.
                        off = 6 + 2 * p if not (tail and p >= 6) else 11 + p
                        actions[base + qc * 16 + off].append(
                            (3, (lambda b=b, qc=qc, p=p: ctx_step(b, qc, p))))
                    # stage rides the same slot as ctx p7 (prio 3.5: right
                    # after it) so psC frees ~1 slot sooner for qc+1.
                    o_st, o_nA, o_nB, o_out = (
                        (20, 22, 24, 27) if not tail else (18, 19, 20, 22))
                    actions[base + qc * 16 + o_st].append(
                        (3.5, (lambda b=b, qc=qc: stage_step(b, qc))))
                    actions[base + qc * 16 + o_nA].append(
                        (2, (lambda b=b, qc=qc: normA_step(b, qc))))
                    actions[base + qc * 16 + o_nB].append(
                        (2, (lambda b=b, qc=qc: normB_step(b, qc))))
                    for st_i in range(4):
                        g = base + qc * 16 + o_out + (2 if not tail else 1) * st_i
                        actions[g].append(
                            (4, (lambda b=b, qc=qc, s=st_i, t=tail:
                                 outp_step(b, qc, s, t))))

            # proj/vaug fillers: batch 0 prologue runs before slot 0; the
            # rest interleave into earlier slots at priority 5.
            def F(step, *a):
                return lambda: step(*a)

            fill0 = {0: F(proj_step, 0, "k", 1), 1: F(proj_step, 0, "k", 2),
                     2: F(vaug_step, 0, 1), 3: F(proj_step, 0, "v", 1),
                     4: F(vaug_step, 0, 2), 5: F(proj_step, 0, "k", 3),
                     6: F(vaug_step, 0, 3), 7: F(proj_step, 0, "v", 2),
                     8: F(vaug_step, 0, 4), 9: F(proj_step, 0, "q", 1),
                     10: F(vaug_step, 0, 5), 11: F(proj_step, 0, "v", 3),
                     12: F(vaug_step, 0, 6), 13: F(vaug_step, 0, 7),
                     14: F(proj_step, 0, "q", 2), 15: F(proj_step, 0, "q", 3)}
            fill1 = {40: F(proj_step, 1, "q", 0), 42: F(proj_step, 1, "k", 0),
                     44: F(proj_step, 1, "v", 0), 46: F(vaug_step, 1, 0),
                     48: F(proj_step, 1, "k", 1), 49: F(proj_step, 1, "k", 2),
                     50: F(vaug_step, 1, 1), 51: F(proj_step, 1, "v", 1),
                     52: F(vaug_step, 1, 2), 53: F(proj_step, 1, "k", 3),
                     54: F(vaug_step, 1, 3), 55: F(proj_step, 1, "v", 2),
                     56: F(vaug_step, 1, 4), 57: F(proj_step, 1, "q", 1),
                     58: F(vaug_step, 1, 5), 59: F(proj_step, 1, "v", 3),
                     60: F(vaug_step, 1, 6), 61: F(vaug_step, 1, 7),
                     62: F(proj_step, 1, "q", 2), 63: F(proj_step, 1, "q", 3)}
            for g, fn in list(fill0.items()) + list(fill1.items()):
                actions[g].append((5, fn))

            # ---- emit: prologue then slots in order ----
            proj_step(0, "q", 0)
            proj_step(0, "k", 0)
            proj_step(0, "v", 0)
            vaug_step(0, 0)
            for g in range(max(actions) + 1):
                for _, fn in sorted(actions[g], key=lambda x: x[0]):
                    fn()

    nc.compile()
    return nc


def _get_nc():
    if "nc" not in _cache:
        _cache["nc"] = _build()
    return _cache["nc"]


def kernel(x, Wq, bq, Wk, bk, Wv, bv, Wo, bo):
    from concourse.bass_utils import run_bass_kernel_spmd

    nc = _get_nc()

    x = np.ascontiguousarray(np.asarray(x, dtype=np.float32))
    xt = np.ascontiguousarray(x.reshape(B * S, D).T)          # [D, B*S]
    idt = np.eye(128, dtype=np.float16)

    def wprep(W, sl, scale=1.0):
        """[1024, 128] slice -> [128, 8*128] with k-tiles side by side:
        out[p, ki*128 + m] = W[ki*128 + p, sl][m] (one contiguous DMA)."""
        w = np.asarray(W, np.float32)[:, sl] * scale
        return np.ascontiguousarray(
            w.reshape(KT_PROJ, 128, HSLICE).transpose(1, 0, 2)
            .reshape(128, KT_PROJ * HSLICE)).astype(np.float16)

    in_maps = []
    for c in range(NCORES):
        sl = slice(c * HSLICE, (c + 1) * HSLICE)
        in_maps.append({
            "xt": xt.astype(np.float16),
            "wq": wprep(Wq, sl, 1.0 / 8.0),
            "wk": wprep(Wk, sl),
            "wv": wprep(Wv, sl),
            "bq": (np.asarray(bq, np.float32)[sl] / 8.0).reshape(HSLICE, 1),
            "bk": np.asarray(bk, np.float32)[sl].reshape(HSLICE, 1),
            "bv": np.asarray(bv, np.float32)[sl].reshape(HSLICE, 1),
            "wo": np.ascontiguousarray(np.asarray(Wo, np.float32)[sl, :]).astype(np.float16),
            "idt": idt,
        })

    res = run_bass_kernel_spmd(nc, in_maps, core_ids=list(range(NCORES)),
                               trace=bool(int(os.environ.get("KTRACE", "0"))))
    _cache["last_result"] = res
    acc = res.results[0]["out"].astype(np.float32)
    for c in range(1, NCORES):
        acc += res.results[c]["out"].astype(np.float32)
    acc += np.asarray(bo, np.float32)[None, :]
    return acc.reshape(B, S, D)
